# revision 5
# baseline (speedup 1.0000x reference)
"""Causal self-attention (B=4, T=2048, C=1024, 16 heads) on 8 Trainium2 cores.

Sharding: batch x head-group.  Core c handles batch b = c//2 and head group
hg = c%2 (8 heads = 4 head-pairs).  Each core computes q/k/v projections for
its heads, causal flash-style attention, and a partial output projection
(its 512 y-channels x full w_proj columns).  The host sums the two partials
per batch (tensor-parallel reduce done host-side at gather).

Per-core kernel (all matmuls fp32r = full-rate reduced-precision fp32):
  phase B1: v projection + PE transposes -> V_wide strips (ones-column packed)
  phase B2: q/k projections -> qT/kT strips [128 (pair ch), t]
  phase C: per (pair, t-tile 512): ST = K@Q^T row-packed head pair; exp on
           ACT (PSUM->SBUF, causal additive mask on the diagonal 128-strip,
           fully-masked columns never computed); y^T accumulated in PSUM with
           the softmax denominator riding in a ones-column of V_wide;
           normalize on eviction with a DMA-broadcast reciprocal; then the
           output projection for that t-tile.
"""
import numpy as np

import concourse.bass as bass
import concourse.tile as tile
from concourse import mybir, bacc
from concourse.bass_utils import run_bass_kernel_spmd

f32 = mybir.dt.float32
f32r = mybir.dt.float32r
Exp = mybir.ActivationFunctionType.Exp

B, T, C = 4, 2048, 1024
N_HEAD = 16
D = C // N_HEAD                 # 64
HPC = N_HEAD // 2               # heads per core = 8
NPAIR = HPC // 2                # head pairs per core = 4
CO_Q = C // 2                   # q channels per core = 512
CT = C // 128                   # contraction tiles for qkv = 8
TJ = T // 512                   # t super-tiles = 4
NS = T // 128                   # s tiles = 16
SCALE = float(D) ** -0.5        # 0.125
NEG = -1.0e30

_CACHE = {}


def _build_nc(reps=1):
    import contextlib
    from concourse.masks import make_identity

    nc = bacc.Bacc("TRN2", target_bir_lowering=False, debug=False)
    xT_d = nc.dram_tensor("xT", [C, T], f32, kind="ExternalInput").ap()
    wqT_d = nc.dram_tensor("wqT", [C, CO_Q], f32, kind="ExternalInput").ap()
    wkT_d = nc.dram_tensor("wkT", [C, CO_Q], f32, kind="ExternalInput").ap()
    wvT_d = nc.dram_tensor("wvT", [C, CO_Q], f32, kind="ExternalInput").ap()
    wpT_d = nc.dram_tensor("wpT", [CO_Q, C], f32, kind="ExternalInput").ap()
    mask_d = nc.dram_tensor("mask", [128, 128], f32, kind="ExternalInput").ap()
    out_d = nc.dram_tensor("out", [T, C], f32, kind="ExternalOutput").ap()
    rscr_d = nc.dram_tensor("rscr", [NPAIR, TJ, 2, 512], f32, kind="Internal").ap()

    with tile.TileContext(nc) as tc:
        for _rep in range(reps):
            _build_body(nc, tc, xT_d, wqT_d, wkT_d, wvT_d, wpT_d, mask_d,
                        out_d, rscr_d)

    nc.compile()
    return nc


def _build_body(nc, tc, xT_d, wqT_d, wkT_d, wvT_d, wpT_d, mask_d, out_d, rscr_d):
    import contextlib
    from concourse.masks import make_identity

    with contextlib.ExitStack() as ctx:
        persist = ctx.enter_context(tc.tile_pool(name="persist", bufs=1))
        qT = persist.tile([128, NPAIR, TJ, 512], f32r)
        kT = persist.tile([128, NPAIR, TJ, 512], f32r)
        vA = persist.tile([128, NPAIR, NS, 128], f32r)
        vB = persist.tile([128, NPAIR, NS, 128], f32r)
        mask = persist.tile([128, 128], f32)
        nc.sync.dma_start(out=mask[:], in_=mask_d[:, :])

        # ---------------- phase B1: v projection + V_wide ----------------
        with (
            tc.tile_pool(name="wv", bufs=1) as wvp,
            tc.tile_pool(name="stg1", bufs=2) as stg1,
            tc.tile_pool(name="xin1", bufs=1) as xin1,
            tc.tile_pool(name="vstg", bufs=2) as vstg,
            tc.tile_pool(name="bps1", bufs=2, space="PSUM") as bps1,
            tc.tile_pool(name="tps", bufs=2, space="PSUM") as tps,
        ):
            onecol = wvp.tile([128, 64], f32)
            nc.vector.memset(onecol[:], 0.0)
            nc.vector.memset(onecol[:, 0:1], 1.0)
            ident = wvp.tile([128, 128], f32)
            make_identity(nc, ident)

            wv_r = wvp.tile([128, CT, CO_Q], f32r)
            for ct in range(CT):
                wstg = stg1.tile([128, CO_Q], f32, tag="stg")
                nc.sync.dma_start(out=wstg[:], in_=wvT_d[ct * 128:(ct + 1) * 128, :])
                nc.vector.tensor_copy(wv_r[:, ct, :], wstg[:])

            for j in range(TJ):
                xr = xin1.tile([128, CT, 512], f32r, tag="xr")
                for ct in range(CT):
                    xstg = stg1.tile([128, 512], f32, tag="stg")
                    nc.sync.dma_start(
                        out=xstg[:],
                        in_=xT_d[ct * 128:(ct + 1) * 128, j * 512:(j + 1) * 512])
                    nc.vector.tensor_copy(xr[:, ct, :], xstg[:])
                for p in range(NPAIR):
                    ps = bps1.tile([128, 512], f32, tag="vps")
                    for ct in range(CT):
                        nc.tensor.matmul(
                            ps[:], wv_r[:, ct, p * 128:(p + 1) * 128], xr[:, ct, :],
                            start=(ct == 0), stop=(ct == CT - 1))
                    vtmp = vstg.tile([128, 512], f32, tag="vtmp")
                    nc.vector.tensor_copy(vtmp[:], ps[:])
                    for sj in range(4):
                        si = j * 4 + sj
                        trp = tps.tile([128, 128], f32, tag="trp")
                        nc.tensor.transpose(
                            trp[:], vtmp[:, sj * 128:(sj + 1) * 128], ident[:])
                        nc.vector.tensor_copy(vA[:, p, si, 0:64], trp[:, 0:64])
                        nc.vector.tensor_copy(vA[:, p, si, 64:128], onecol[:])
                        nc.vector.tensor_copy(vB[:, p, si, 0:64], onecol[:])
                        nc.vector.tensor_copy(vB[:, p, si, 64:128], trp[:, 64:128])

        # ---------------- phase B2: q/k projections ----------------
        with (
            tc.tile_pool(name="wqk", bufs=1) as wqkp,
            tc.tile_pool(name="stg2", bufs=2) as stg2,
            tc.tile_pool(name="xin2", bufs=1) as xin2,
            tc.tile_pool(name="bps2", bufs=2, space="PSUM") as bps2,
        ):
            w_r = wqkp.tile([128, CT, 2, CO_Q], f32r)
            for i, wd in enumerate((wqT_d, wkT_d)):
                for ct in range(CT):
                    wstg = stg2.tile([128, CO_Q], f32, tag="stg")
                    nc.sync.dma_start(out=wstg[:], in_=wd[ct * 128:(ct + 1) * 128, :])
                    nc.vector.tensor_copy(w_r[:, ct, i, :], wstg[:])

            for j in range(TJ):
                xr = xin2.tile([128, CT, 512], f32r, tag="xr")
                for ct in range(CT):
                    xstg = stg2.tile([128, 512], f32, tag="stg")
                    nc.sync.dma_start(
                        out=xstg[:],
                        in_=xT_d[ct * 128:(ct + 1) * 128, j * 512:(j + 1) * 512])
                    nc.vector.tensor_copy(xr[:, ct, :], xstg[:])
                for i, dst in ((0, qT), (1, kT)):
                    for p in range(NPAIR):
                        ps = bps2.tile([128, 512], f32, tag="qkps")
                        for ct in range(CT):
                            nc.tensor.matmul(
                                ps[:], w_r[:, ct, i, p * 128:(p + 1) * 128],
                                xr[:, ct, :],
                                start=(ct == 0), stop=(ct == CT - 1))
                        nc.vector.tensor_copy(dst[:, p, j, :], ps[:])

        # ---------------- phase C: attention + projection ----------------
        with (
            tc.tile_pool(name="wp2", bufs=1) as wp2,
            tc.tile_pool(name="stg3", bufs=2) as stg3,
            tc.tile_pool(name="pw", bufs=2) as pw,
            tc.tile_pool(name="yb", bufs=2) as yb,
            tc.tile_pool(name="ob", bufs=2) as ob,
            tc.tile_pool(name="rbp", bufs=2) as rbp,
            tc.tile_pool(name="yps", bufs=1, space="PSUM") as yps_pool,
            tc.tile_pool(name="sps", bufs=2, space="PSUM") as sps,
            tc.tile_pool(name="pps", bufs=2, space="PSUM") as pps,
        ):
            wpT_r = wp2.tile([128, NPAIR, C], f32r)
            for p in range(NPAIR):
                wstg2 = stg3.tile([128, C], f32, tag="wstg2")
                nc.sync.dma_start(out=wstg2[:], in_=wpT_d[p * 128:(p + 1) * 128, :])
                nc.vector.tensor_copy(wpT_r[:, p, :], wstg2[:])

            for j in range(TJ):
                Y = yb.tile([128, NPAIR, 512], f32r, tag="Y")
                for p in range(NPAIR):
                    ypsA = yps_pool.tile([128, 512], f32, tag="ypsA")
                    ypsB = yps_pool.tile([128, 512], f32, tag="ypsB")
                    nsj = 4 * (j + 1)
                    for si in range(nsj):
                        rel = si * 128 - j * 512
                        lo = max(rel, 0)
                        stA = sps.tile([128, 512], f32, tag="stA")
                        stB = sps.tile([128, 512], f32, tag="stB")
                        kslc = (si // 4, slice((si % 4) * 128, (si % 4) * 128 + 128))
                        nc.tensor.matmul(
                            stA[:, lo:], kT[0:64, p, kslc[0], kslc[1]],
                            qT[0:64, p, j, lo:], start=True, stop=True)
                        nc.tensor.matmul(
                            stB[:, lo:], kT[64:128, p, kslc[0], kslc[1]],
                            qT[64:128, p, j, lo:], start=True, stop=True)
                        if rel >= 0:   # diagonal block: additive causal mask
                            nc.vector.tensor_add(
                                stA[:, lo:lo + 128], stA[:, lo:lo + 128], mask[:])
                            nc.vector.tensor_add(
                                stB[:, lo:lo + 128], stB[:, lo:lo + 128], mask[:])
                        pA = pw.tile([128, 512], f32r, tag="pA")
                        pB = pw.tile([128, 512], f32r, tag="pB")
                        nc.scalar.activation(pA[:, lo:], stA[:, lo:], Exp, scale=SCALE)
                        nc.scalar.activation(pB[:, lo:], stB[:, lo:], Exp, scale=SCALE)
                        st = (si == 0)
                        sp = (si == nsj - 1)
                        nc.tensor.matmul(ypsA[:, lo:], vA[:, p, si, :], pA[:, lo:],
                                         start=st, stop=sp)
                        nc.tensor.matmul(ypsB[:, lo:], vB[:, p, si, :], pB[:, lo:],
                                         start=st, stop=sp)
                    # normalize: lA at ypsA row 64, lB at ypsB row 0
                    r = rbp.tile([64, 512], f32, tag="r")
                    nc.vector.reciprocal(r[0:1, :], ypsA[64:65, :])
                    nc.vector.reciprocal(r[32:33, :], ypsB[0:1, :])
                    nc.sync.dma_start(out=rscr_d[p, j, 0:1, :], in_=r[0:1, :])
                    nc.sync.dma_start(out=rscr_d[p, j, 1:2, :], in_=r[32:33, :])
                    rb = rbp.tile([128, 512], f32, tag="rb")
                    nc.sync.dma_start(
                        out=rb[0:64, :],
                        in_=rscr_d[p, j, 0:1, :].partition_broadcast(64))
                    nc.sync.dma_start(
                        out=rb[64:128, :],
                        in_=rscr_d[p, j, 1:2, :].partition_broadcast(64))
                    nc.vector.tensor_mul(Y[0:64, p, :], ypsA[0:64, :], rb[0:64, :])
                    nc.vector.tensor_mul(Y[64:128, p, :], ypsB[64:128, :], rb[64:128, :])

                # output projection for this t super-tile
                for tj in range(4):
                    o_sb = ob.tile([128, C], f32, tag="o")
                    for nh in range(2):
                        prps = pps.tile([128, 512], f32, tag="prps")
                        for p in range(NPAIR):
                            nc.tensor.matmul(
                                prps[:], Y[:, p, tj * 128:(tj + 1) * 128],
                                wpT_r[:, p, nh * 512:(nh + 1) * 512],
                                start=(p == 0), stop=(p == NPAIR - 1))
                        nc.vector.tensor_copy(o_sb[:, nh * 512:(nh + 1) * 512], prps[:])
                    row = j * 512 + tj * 128
                    nc.sync.dma_start(out=out_d[row:row + 128, :], in_=o_sb[:])


def _get_nc(reps=1):
    key = f"nc{reps}"
    if key not in _CACHE:
        _CACHE[key] = _build_nc(reps)
    return _CACHE[key]


def make_in_maps(x, w_qkv, w_proj):
    """Shard full inputs into the 8 per-core input maps."""
    x = np.asarray(x, dtype=np.float32)
    w_qkv = np.asarray(w_qkv, dtype=np.float32)
    w_proj = np.asarray(w_proj, dtype=np.float32)
    mask = np.where(np.arange(128)[:, None] <= np.arange(128)[None, :],
                    np.float32(0.0), np.float32(NEG)).astype(np.float32)
    in_maps = []
    for c in range(8):
        b, hg = c // 2, c % 2
        sl = slice(hg * CO_Q, (hg + 1) * CO_Q)
        in_maps.append({
            "xT": np.ascontiguousarray(x[b].T),
            "wqT": np.ascontiguousarray(w_qkv[0 * C:1 * C][sl].T),
            "wkT": np.ascontiguousarray(w_qkv[1 * C:2 * C][sl].T),
            "wvT": np.ascontiguousarray(w_qkv[2 * C:3 * C][sl].T),
            "wpT": np.ascontiguousarray(w_proj[:, sl].T),
            "mask": mask,
        })
    return in_maps


def gather(results):
    """Sum the two head-group partials per batch, stack batches."""
    out = np.empty((B, T, C), dtype=np.float32)
    for b in range(B):
        out[b] = results[2 * b]["out"] + results[2 * b + 1]["out"]
    return out


def kernel(x, w_qkv, w_proj):
    nc = _get_nc()
    in_maps = make_in_maps(x, w_qkv, w_proj)
    res = run_bass_kernel_spmd(nc, in_maps, core_ids=list(range(8)))
    return gather(res.results)
